# revision 12
# baseline (speedup 1.0000x reference)
"""Trainium2 Bass kernel for nn_DagnabbitAutoEncoder (gnn_message_passing).

Self-contained: kernel(**inputs) takes FULL inputs, returns FULL [B,N,D]
output. Data-parallel over graphs across 8 NeuronCores; the DAG scan is
converted into ~24 level-wavefronts on the host.

v3 design (vs the scatter-based v2 baseline):
- The node-embedding buffer lives in DRAM as bf16 rows of 256B stride:
  [e (64 bf16) ; zeros (64 bf16)], laid out LEVEL-SORTED so every level's
  outputs form one contiguous block -> the "scatter" becomes a plain
  HWDGE dma_start on SP (cheap), eliminating all scatter descriptors.
- Parent fetch uses the TRANSPOSED dma_gather (elem=128 bf16 = 256B):
  fetch k lands feature-major in column k of X [128, 2S] (parent0 cols
  [0,S), parent1 cols [S,2S)). This kills the PE transpose, the f32->bf16
  cast and the xT PSUM drain of the classic pipeline.
- MM1 = two accumulating matmuls per type segment with half-zeroed
  weights (lhsT [W1a;0] on X0-cols, [W1b;0] on X1-cols) since each
  gathered column only has valid features on partitions 0..63 (the junk
  half of each 256B row is runtime-zeroed DRAM x zero weights).
- Type segments are padded to 32 slots (matmul output base partition must
  be 0/32/64/96), rows/osb use this 32-padded layout; gather/MM1/gelu run
  on the packed layout. Per-(u,l,t) counts are max'd over the 8 cores so
  the program is SPMD-shared; per-core shortfall slots fetch row 0.
"""

B_, N_, R_, D_, K_, T_, M_ = 256, 2048, 64, 64, 2, 8, 8

import numpy as np

UNIT_SIZES = [11, 11, 10]
HT_GROUP = 1024  # hT tile cols (2 PSUM banks); MM pieces split at 512 lines
OPS_CHUNKS = 8  # o_ps tile holds 8 chunks of 64 cols (1 bank)


def compute_levels(idx, R, N):
    B = idx.shape[0]
    lvl = np.zeros((B, N), np.int32)
    ar = np.arange(B)
    for i in range(R, N):
        lvl[:, i] = 1 + lvl[ar[:, None], idx[:, i, :]].max(axis=1)
    return lvl


def _wrap16(vals, ncols):
    """vals [n] -> [128, ncols] int16 wrapped-16 + replicated layout."""
    n = len(vals)
    arr = np.zeros((16, ncols), np.int16)
    k = np.arange(n)
    arr[k % 16, k // 16] = vals
    return np.tile(arr, (8, 1))


def _r(x, m):
    return -(-x // m) * m


def build_schedule(idx, types, B, N, R, T, M, pad=64):
    BL = B // M
    unit_sizes = UNIT_SIZES
    NU = len(unit_sizes)
    ubase = np.concatenate([[0], np.cumsum(unit_sizes)])
    lvl = compute_levels(idx, R, N)
    L = int(lvl[:, R:].max())
    types_np = np.asarray(types)
    idx_np = np.asarray(idx)

    # per (core, unit, level, type): node lists (in (b, i) order)
    nodes = [[[[None] * T for _ in range(L + 1)] for _ in range(NU)] for _ in range(M)]
    cnt = np.zeros((M, NU, L + 1, T), np.int64)
    for m in range(M):
        for u in range(NU):
            g0 = m * BL + ubase[u]
            gu = unit_sizes[u]
            lv = lvl[g0 : g0 + gu, R:]
            tp = types_np[g0 : g0 + gu, R:]
            for l in range(1, L + 1):
                for t in range(T):
                    bb, ii = np.nonzero((lv == l) & (tp == t))
                    nodes[m][u][l][t] = (bb, ii + R)
                    cnt[m, u, l, t] = len(bb)
    maxcnt = cnt.max(axis=0)  # [NU, L+1, T]

    # shared geometry per (u, l): packed offsets o_t, 32-padded offsets q_t
    o_off = np.zeros((NU, L + 1, T), np.int64)
    q_off = np.zeros((NU, L + 1, T), np.int64)
    S_pack = np.zeros((NU, L + 1), np.int64)  # gather slots (64-rounded)
    S_pad = np.zeros((NU, L + 1), np.int64)  # osb/row slots (128-rounded)
    for u in range(NU):
        for l in range(1, L + 1):
            o = q = 0
            for t in range(T):
                o_off[u, l, t] = o
                q_off[u, l, t] = q
                o += maxcnt[u, l, t]
                q += _r(int(maxcnt[u, l, t]), pad)
            S_pack[u, l] = _r(o, 64)
            S_pad[u, l] = _r(q, 128)

    # row bases: root block (128-rounded) then level blocks
    root_rows = [_r(unit_sizes[u] * R, 128) for u in range(NU)]
    B_ul = np.zeros((NU, L + 1), np.int64)
    rows_u = []
    for u in range(NU):
        r = root_rows[u]
        for l in range(1, L + 1):
            B_ul[u, l] = r
            r += S_pad[u, l]
        rows_u.append(int(r))
    if max(rows_u) > 32767 and pad > 32:
        return build_schedule(idx, types, B, N, R, T, M, pad=32)
    assert max(rows_u) <= 32767, f"rows {rows_u} exceed int16"

    # per-core node->row map and gather idx streams
    gcol = np.zeros((NU, L + 1), np.int64)
    w = 0
    for l in range(1, L + 1):
        for u in range(NU):
            gcol[u, l] = w
            w += (2 * int(S_pack[u, l])) // 16
    W16 = max(int(w), 1)

    idx16_per_core = []
    for m in range(M):
        # row of node (b_local, i) within its unit
        row_of = [np.zeros((unit_sizes[u], N), np.int64) for u in range(NU)]
        for u in range(NU):
            gu = unit_sizes[u]
            bl = np.arange(gu)
            row_of[u][:, :R] = bl[:, None] * R + np.arange(R)[None, :]
            for l in range(1, L + 1):
                C = int(S_pad[u, l]) // 128
                for t in range(T):
                    bb, ii = nodes[m][u][l][t]
                    s = q_off[u, l, t] + np.arange(len(bb))
                    # partition-major rows: slot s -> row (s%128)*C + s//128
                    row_of[u][bb, ii] = B_ul[u, l] + (s % 128) * C + s // 128
        arr = np.zeros((128, W16), np.int16)
        for u in range(NU):
            g0 = m * BL + ubase[u]
            for l in range(1, L + 1):
                S = int(S_pack[u, l])
                if S == 0:
                    continue
                gv = np.zeros(2 * S, np.int64)
                for t in range(T):
                    bb, ii = nodes[m][u][l][t]
                    o = int(o_off[u, l, t])
                    s = o + np.arange(len(bb))
                    gv[s] = row_of[u][bb, idx_np[g0 + bb, ii, 0]]
                    gv[S + s] = row_of[u][bb, idx_np[g0 + bb, ii, 1]]
                arr[:, gcol[u, l] : gcol[u, l] + (2 * S) // 16] = _wrap16(
                    gv, (2 * S) // 16
                )
        idx16_per_core.append(arr)

    return dict(
        L=L,
        BL=BL,
        NU=NU,
        unit_sizes=unit_sizes,
        ubase=ubase,
        maxcnt=maxcnt,
        o_off=o_off,
        q_off=q_off,
        S_pack=S_pack,
        S_pad=S_pad,
        B_ul=B_ul,
        rows_u=rows_u,
        root_rows=root_rows,
        gcol=gcol,
        W16=W16,
        idx16_per_core=idx16_per_core,
        nodes=nodes,
    )


def build_inputs(root_embeddings, W1, b1, W2, b2, sched, N, R, D, T, M):
    import ml_dtypes

    bf = ml_dtypes.bfloat16
    BL = sched["BL"]
    NU = sched["NU"]
    unit_sizes = sched["unit_sizes"]
    ubase = sched["ubase"]
    W1n = np.asarray(W1, np.float32)  # [T, 128, 128]
    # w1a[t] = [W1[t][0:64,:]; 0], w1b[t] = [W1[t][64:128,:]; 0]
    w1a = np.zeros((128, T * 128), np.float32)
    w1b = np.zeros((128, T * 128), np.float32)
    for t in range(T):
        w1a[0:64, t * 128 : (t + 1) * 128] = W1n[t, 0:64, :]
        w1b[0:64, t * 128 : (t + 1) * 128] = W1n[t, 64:128, :]
    w2 = np.ascontiguousarray(
        np.transpose(np.asarray(W2, np.float32), (1, 0, 2)).reshape(128, T * D)
    )
    b1a = np.ascontiguousarray(np.asarray(b1, np.float32).T)  # [128, T]
    b2f = np.broadcast_to(
        np.asarray(b2, np.float32).reshape(1, T * D), (128, T * D)
    ).copy()

    roots_np = np.asarray(root_embeddings, np.float32)
    in_maps = []
    for m in range(M):
        roots3 = np.zeros((2304, D), np.float32)
        pos = 0
        for u in range(NU):
            gu = unit_sizes[u]
            g0 = m * BL + ubase[u]
            blk = roots_np[g0 : g0 + gu].reshape(gu * R, D)
            roots3[pos : pos + gu * R] = blk
            pos += 768
        in_maps.append(
            dict(
                roots3=roots3,
                w1a=w1a.astype(bf),
                w1b=w1b.astype(bf),
                w2=w2.astype(bf),
                b1a=b1a,
                b2f=b2f,
                idx16=sched["idx16_per_core"][m],
            )
        )
    return in_maps


def _cap_waits(nc, max_waits=1):
    import concourse.mybir as mb

    k = 0
    for f in nc.m.functions:
        for bb in f.blocks:
            out = []
            for ins in bb.instructions:
                si = getattr(ins, "sync_info", None)
                if si is not None and si.on_wait and len(si.on_wait) > max_waits:
                    waits = list(si.on_wait)
                    keep = waits[:max_waits]
                    for wv in waits[max_waits:]:
                        nop = mb.InstNoOp(name=f"waitnop_{k}", ins=[], outs=[])
                        k += 1
                        nop.engine = ins.engine
                        nop.sync_info = mb.SyncInfo(on_wait=[wv], on_update=[])
                        out.append(nop)
                    ins.sync_info = mb.SyncInfo(
                        on_wait=keep, on_update=list(si.on_update or [])
                    )
                out.append(ins)
            bb.instructions = out
    return k


def build_program(sched, N, R, D, T, split_waits=True, zero_b1=True, zero_b2=True):
    import concourse.bass as bass
    import concourse.mybir as mybir
    from concourse import tile
    from concourse.tile_rust import add_dep_helper

    L = sched["L"]
    NU = sched["NU"]
    unit_sizes = sched["unit_sizes"]
    maxcnt = sched["maxcnt"]
    o_off = sched["o_off"]
    q_off = sched["q_off"]
    S_pack = sched["S_pack"]
    S_pad = sched["S_pad"]
    B_ul = sched["B_ul"]
    rows_u = sched["rows_u"]
    root_rows = sched["root_rows"]
    gcol = sched["gcol"]
    W16 = sched["W16"]
    f32 = mybir.dt.float32
    bf16 = mybir.dt.bfloat16
    i16 = mybir.dt.int16

    nc = bass.Bass()
    roots3 = nc.declare_dram_parameter("roots3", [2304, D], f32, isOutput=False)
    w1a = nc.declare_dram_parameter("w1a", [128, T * 128], bf16, isOutput=False)
    w1b = nc.declare_dram_parameter("w1b", [128, T * 128], bf16, isOutput=False)
    w2 = nc.declare_dram_parameter("w2", [128, T * D], bf16, isOutput=False)
    b1a = nc.declare_dram_parameter("b1a", [128, T], f32, isOutput=False)
    b2f = nc.declare_dram_parameter("b2f", [128, T * D], f32, isOutput=False)
    idx16 = nc.declare_dram_parameter("idx16", [128, W16], i16, isOutput=False)
    bufs = [
        nc.declare_dram_parameter(f"buf{u}", [rows_u[u], 128], bf16, isOutput=True)
        for u in range(NU)
    ]

    Spack_max = [int(S_pack[u, 1:].max()) for u in range(NU)]
    Cmax = [int(S_pad[u, 1:].max()) // 128 for u in range(NU)]

    with tile.TileContext(nc) as tc:
        with (
            tc.tile_pool(name="const", bufs=1) as constp,
            tc.tile_pool(name="lvl", bufs=2) as lvlp,
            tc.tile_pool(name="hts", bufs=3) as htsp,
            tc.tile_pool(name="ps", bufs=3, space="PSUM") as psp,
            tc.tile_pool(name="pso", bufs=2, space="PSUM") as psop,
        ):
            from concourse import library_config

            nc.gpsimd.load_library(library_config.mlp)
            w1asb = constp.tile([128, T * 128], bf16)
            nc.sync.dma_start(out=w1asb[:], in_=w1a[:])
            w1bsb = constp.tile([128, T * 128], bf16)
            nc.sync.dma_start(out=w1bsb[:], in_=w1b[:])
            w2sb = constp.tile([128, T * D], bf16)
            nc.sync.dma_start(out=w2sb[:], in_=w2[:])
            b1sb = constp.tile([128, T], f32)
            nc.sync.dma_start(out=b1sb[:], in_=b1a[:])
            b2fsb = constp.tile([128, T * D], f32)
            nc.sync.dma_start(out=b2fsb[:], in_=b2f[:])
            idxsb = constp.tile([128, W16], i16)
            nc.sync.dma_start(out=idxsb[:], in_=idx16[:])

            # roots: [2304,64] f32 -> SBUF -> bf16 -> per-unit payload store
            rsb = constp.tile([128, 18, D], f32)
            nc.sync.dma_start(
                out=rsb[:], in_=roots3[:].rearrange("(c p) d -> p c d", p=128)
            )
            rsbb = constp.tile([128, 18, D], bf16)
            nc.vector.tensor_copy(
                rsbb[:].rearrange("p c d -> p (c d)"),
                rsb[:].rearrange("p c d -> p (c d)"),
            )
            root_stores = []
            for u in range(NU):
                nc_cols = root_rows[u] // 128
                rs = nc.sync.dma_start(
                    out=bufs[u][0 : root_rows[u], 0:D].rearrange(
                        "(c p) d -> p c d", p=128
                    ),
                    in_=rsbb[:, 6 * u : 6 * u + nc_cols, :],
                )
                root_stores.append(rs)

            _regcache = {}

            def creg_for(v):
                if v not in _regcache:
                    _regcache[v] = nc.gpsimd.to_reg(v)
                return _regcache[v]

            prev_stores = [[rs] for rs in root_stores]
            prev_gather = [None] * NU

            for l in range(1, L + 1):
                xs = {}
                gathers = {}
                for u in range(NU):
                    S = int(S_pack[u, l])
                    if S == 0:
                        continue
                    x = lvlp.tile([128, 2 * Spack_max[u]], bf16, tag=f"x{u}")
                    xs[u] = (x, S)
                    g = nc.gpsimd.dma_gather(
                        out_ap=x[:, 0 : 2 * S].rearrange(
                            "p (a s) -> p a s", a=1
                        ),
                        in_ap=bufs[u][:],
                        idxs_ap=idxsb[:, gcol[u, l] : gcol[u, l] + (2 * S) // 16],
                        num_idxs=2 * S,
                        num_idxs_reg=creg_for(2 * S),
                        elem_size=128,
                        transpose=True,
                        single_packet=False,
                    )
                    for ps_h in prev_stores[u]:
                        add_dep_helper(
                            g.ins, ps_h.ins, sync=True, reason="lvl order"
                        )
                    if prev_gather[u] is not None:
                        add_dep_helper(
                            g.ins, prev_gather[u].ins, sync=True, reason="chain"
                        )
                    prev_gather[u] = g
                    gathers[u] = g

                for u in range(NU):
                    if u not in xs:
                        continue
                    x, S = xs[u]
                    SP = int(S_pad[u, l])
                    C = SP // 128
                    osb = lvlp.tile([128, Cmax[u] * D], bf16, tag=f"osb{u}")
                    lvl_stores = []
                    buf_blk = bufs[u][
                        int(B_ul[u, l]) : int(B_ul[u, l]) + C * 128, 0:D
                    ].rearrange("(p c) d -> p c d", c=C)

                    # segments: (t, o, q, n) with n = maxcnt
                    segs = [
                        (
                            t,
                            int(o_off[u, l, t]),
                            int(q_off[u, l, t]),
                            int(maxcnt[u, l, t]),
                        )
                        for t in range(T)
                        if maxcnt[u, l, t] > 0
                    ]
                    q_end = segs[-1][2] + segs[-1][3]

                    # process in hT groups of HT_GROUP padded cols
                    # (HT_GROUP == OPS_CHUNKS*128 so one o_ps tile per group)
                    ngr = -(-q_end // HT_GROUP)
                    for gi in range(ngr):
                        g0 = gi * HT_GROUP
                        g1 = min(g0 + HT_GROUP, q_end)
                        span = g1 - g0
                        hT_ps = psp.tile([128, HT_GROUP], f32, tag="hT")
                        hTs = htsp.tile([128, HT_GROUP], bf16, tag="hTs")
                        o_ps = psop.tile([128, OPS_CHUNKS * D], f32, tag="o")
                        # MM1: per segment piece within [g0,g1), split at
                        # 512 lines (PSUM bank) in local col space
                        for t, o, q, n in segs:
                            lo = max(q, g0)
                            hi = min(q + n, g1)
                            while lo < hi:
                                nxt = min(hi, g0 + ((lo - g0) // 512 + 1) * 512)
                                a = lo - g0
                                b = nxt - g0
                                po = o + (lo - q)
                                pn = nxt - lo
                                nc.tensor.matmul(
                                    hT_ps[:, a:b],
                                    lhsT=w1asb[:, t * 128 : (t + 1) * 128],
                                    rhs=x[:, po : po + pn],
                                    start=True,
                                    stop=False,
                                )
                                nc.tensor.matmul(
                                    hT_ps[:, a:b],
                                    lhsT=w1bsb[:, t * 128 : (t + 1) * 128],
                                    rhs=x[:, S + po : S + po + pn],
                                    start=False,
                                    stop=True,
                                )
                                lo = nxt
                        # gelu over the whole group span (pads = junk)
                        if zero_b1:
                            nc.scalar.activation(
                                hTs[:, 0:span],
                                hT_ps[:, 0:span],
                                mybir.ActivationFunctionType.Gelu,
                                bias=b1sb[:, 0:1],
                            )
                        else:
                            for t, o, q, n in segs:
                                lo = max(q, g0)
                                hi = min(q + n, g1)
                                if lo < hi:
                                    nc.scalar.activation(
                                        hTs[:, lo - g0 : hi - g0],
                                        hT_ps[:, lo - g0 : hi - g0],
                                        mybir.ActivationFunctionType.Gelu,
                                        bias=b1sb[:, t : t + 1],
                                    )
                        # MM2: pieces within this group into o_ps; base
                        # partition tiling: 0 -> any, 64 -> <=64,
                        # 32/96 -> <=32 (PE tile grid)
                        for t, o, q, n in segs:
                            lo = max(q, g0)
                            hi = min(q + n, g1)
                            while lo < hi:
                                c = lo // 128
                                a = lo % 128
                                nb = 64 if a == 32 else 128
                                nxt = min(hi, c * 128 + nb)
                                pn = nxt - lo
                                cc = c % OPS_CHUNKS
                                nc.tensor.matmul(
                                    o_ps[a : a + pn, cc * D : (cc + 1) * D],
                                    lhsT=hTs[:, lo - g0 : nxt - g0],
                                    rhs=w2sb[:, t * D : (t + 1) * D],
                                    start=True,
                                    stop=True,
                                )
                                lo = nxt
                        # drain this group's o_ps -> osb (bf16)
                        c0 = g0 // 128
                        nch = min(-(-span // 128), C - c0)
                        if zero_b2:
                            nc.vector.tensor_copy(
                                osb[:, c0 * D : (c0 + nch) * D],
                                o_ps[:, 0 : nch * D],
                            )
                        else:
                            for t, o, q, n in segs:
                                lo = max(q, g0)
                                hi = min(q + n, g1)
                                while lo < hi:
                                    c = lo // 128
                                    a = lo % 128
                                    nxt = min(hi, (c + 1) * 128)
                                    pn = nxt - lo
                                    cc = c % OPS_CHUNKS
                                    nc.vector.tensor_tensor(
                                        out=osb[a : a + pn, c * D : c * D + D],
                                        in0=o_ps[
                                            a : a + pn, cc * D : cc * D + D
                                        ],
                                        in1=b2fsb[
                                            a : a + pn, t * D : (t + 1) * D
                                        ],
                                        op=mybir.AluOpType.add,
                                    )
                                    lo = nxt
                        # store this group's chunks (partition-major rows)
                        st = nc.sync.dma_start(
                            out=buf_blk[:, c0 : c0 + nch, :],
                            in_=osb[:, c0 * D : (c0 + nch) * D].rearrange(
                                "p (c d) -> p c d", d=D
                            ),
                        )
                        add_dep_helper(
                            st.ins, gathers[u].ins, sync=True, reason="war"
                        )
                        lvl_stores.append(st)
                    prev_stores[u] = lvl_stores

    from concourse.library_overlay import lower_extended_insts

    lower_extended_insts(nc)
    if split_waits:
        _cap_waits(nc)
    return nc


def assemble_output(results, sched, roots_np, N, R, D, M):
    L = sched["L"]
    NU = sched["NU"]
    BL = sched["BL"]
    unit_sizes = sched["unit_sizes"]
    ubase = sched["ubase"]
    q_off = sched["q_off"]
    B_ul = sched["B_ul"]
    nodes = sched["nodes"]
    out = np.zeros((M * BL, N, D), np.float32)
    out[:, :R] = roots_np
    for m in range(M):
        for u in range(NU):
            buf = np.asarray(results[m][f"buf{u}"]).astype(np.float32)
            g0 = m * BL + ubase[u]
            for l in range(1, L + 1):
                C = int(sched["S_pad"][u, l]) // 128
                for t in range(8):
                    bb, ii = nodes[m][u][l][t]
                    if len(bb) == 0:
                        continue
                    s = int(q_off[u, l, t]) + np.arange(len(bb))
                    rows = int(B_ul[u, l]) + (s % 128) * C + s // 128
                    out[g0 + bb, ii] = buf[rows, 0:D]
    return out


def kernel(**inputs):
    import numpy as np

    root_embeddings = np.asarray(inputs["root_embeddings"], np.float32)
    W1 = np.asarray(inputs["W1"], np.float32)
    b1 = np.asarray(inputs["b1"], np.float32)
    W2 = np.asarray(inputs["W2"], np.float32)
    b2 = np.asarray(inputs["b2"], np.float32)
    idx = np.asarray(inputs["node_inputs_indices"], np.int32)
    types = np.asarray(inputs["node_types"], np.int32)

    B, N, R, D, T, M = B_, N_, R_, D_, T_, M_
    sched = build_schedule(idx, types, B, N, R, T, M)
    in_maps = build_inputs(root_embeddings, W1, b1, W2, b2, sched, N, R, D, T, M)
    nc = build_program(
        sched, N, R, D, T,
        zero_b1=not np.any(b1),
        zero_b2=not np.any(b2),
    )

    from concourse.bass_utils import run_bass_kernel_spmd

    res = run_bass_kernel_spmd(nc, in_maps, core_ids=list(range(M)))
    out = assemble_output(res.results, sched, root_embeddings, N, R, D, M)
    return out.astype(np.float32)


# revision 13
# speedup vs baseline: 1.0160x; 1.0160x over previous
"""Trainium2 Bass kernel for nn_DagnabbitAutoEncoder (gnn_message_passing).

Self-contained: kernel(**inputs) takes FULL inputs, returns FULL [B,N,D]
output. Data-parallel over graphs across 8 NeuronCores; the DAG scan is
converted into ~24 level-wavefronts on the host.

v3 design (vs the scatter-based v2 baseline):
- The node-embedding buffer lives in DRAM as bf16 rows of 256B stride:
  [e (64 bf16) ; zeros (64 bf16)], laid out LEVEL-SORTED so every level's
  outputs form one contiguous block -> the "scatter" becomes a plain
  HWDGE dma_start on SP (cheap), eliminating all scatter descriptors.
- Parent fetch uses the TRANSPOSED dma_gather (elem=128 bf16 = 256B):
  fetch k lands feature-major in column k of X [128, 2S] (parent0 cols
  [0,S), parent1 cols [S,2S)). This kills the PE transpose, the f32->bf16
  cast and the xT PSUM drain of the classic pipeline.
- MM1 = two accumulating matmuls per type segment with half-zeroed
  weights (lhsT [W1a;0] on X0-cols, [W1b;0] on X1-cols) since each
  gathered column only has valid features on partitions 0..63 (the junk
  half of each 256B row is runtime-zeroed DRAM x zero weights).
- Type segments are padded to 32 slots (matmul output base partition must
  be 0/32/64/96), rows/osb use this 32-padded layout; gather/MM1/gelu run
  on the packed layout. Per-(u,l,t) counts are max'd over the 8 cores so
  the program is SPMD-shared; per-core shortfall slots fetch row 0.
"""

B_, N_, R_, D_, K_, T_, M_ = 256, 2048, 64, 64, 2, 8, 8

import numpy as np

UNIT_SIZES = [11, 11, 10]
HT_GROUP = 1024  # hT tile cols (2 PSUM banks); MM pieces split at 512 lines
OPS_CHUNKS = 8  # o_ps tile holds 8 chunks of 64 cols (1 bank)


def compute_levels(idx, R, N):
    B = idx.shape[0]
    lvl = np.zeros((B, N), np.int32)
    ar = np.arange(B)
    for i in range(R, N):
        lvl[:, i] = 1 + lvl[ar[:, None], idx[:, i, :]].max(axis=1)
    return lvl


def _wrap16(vals, ncols):
    """vals [n] -> [128, ncols] int16 wrapped-16 + replicated layout."""
    n = len(vals)
    arr = np.zeros((16, ncols), np.int16)
    k = np.arange(n)
    arr[k % 16, k // 16] = vals
    return np.tile(arr, (8, 1))


def _r(x, m):
    return -(-x // m) * m


def build_schedule(idx, types, B, N, R, T, M, pad=64):
    BL = B // M
    unit_sizes = UNIT_SIZES
    NU = len(unit_sizes)
    ubase = np.concatenate([[0], np.cumsum(unit_sizes)])
    lvl = compute_levels(idx, R, N)
    L = int(lvl[:, R:].max())
    types_np = np.asarray(types)
    idx_np = np.asarray(idx)

    # per (core, unit, level, type): node lists (in (b, i) order)
    nodes = [[[[None] * T for _ in range(L + 1)] for _ in range(NU)] for _ in range(M)]
    cnt = np.zeros((M, NU, L + 1, T), np.int64)
    for m in range(M):
        for u in range(NU):
            g0 = m * BL + ubase[u]
            gu = unit_sizes[u]
            lv = lvl[g0 : g0 + gu, R:]
            tp = types_np[g0 : g0 + gu, R:]
            for l in range(1, L + 1):
                for t in range(T):
                    bb, ii = np.nonzero((lv == l) & (tp == t))
                    nodes[m][u][l][t] = (bb, ii + R)
                    cnt[m, u, l, t] = len(bb)
    maxcnt = cnt.max(axis=0)  # [NU, L+1, T]

    # shared geometry per (u, l): packed offsets o_t, 32-padded offsets q_t
    o_off = np.zeros((NU, L + 1, T), np.int64)
    q_off = np.zeros((NU, L + 1, T), np.int64)
    S_pack = np.zeros((NU, L + 1), np.int64)  # gather slots (64-rounded)
    S_pad = np.zeros((NU, L + 1), np.int64)  # osb/row slots (128-rounded)
    for u in range(NU):
        for l in range(1, L + 1):
            o = q = 0
            for t in range(T):
                o_off[u, l, t] = o
                q_off[u, l, t] = q
                o += maxcnt[u, l, t]
                q += _r(int(maxcnt[u, l, t]), pad)
            S_pack[u, l] = _r(o, 64)
            S_pad[u, l] = _r(q, 128)

    # row bases: root block (128-rounded) then level blocks
    root_rows = [_r(unit_sizes[u] * R, 128) for u in range(NU)]
    B_ul = np.zeros((NU, L + 1), np.int64)
    rows_u = []
    for u in range(NU):
        r = root_rows[u]
        for l in range(1, L + 1):
            B_ul[u, l] = r
            r += S_pad[u, l]
        rows_u.append(int(r))
    if max(rows_u) > 32767 and pad > 32:
        return build_schedule(idx, types, B, N, R, T, M, pad=32)
    assert max(rows_u) <= 32767, f"rows {rows_u} exceed int16"

    # per-core node->row map and gather idx streams
    gcol = np.zeros((NU, L + 1), np.int64)
    w = 0
    for l in range(1, L + 1):
        for u in range(NU):
            gcol[u, l] = w
            w += (2 * int(S_pack[u, l])) // 16
    W16 = max(int(w), 1)

    idx16_per_core = []
    for m in range(M):
        # row of node (b_local, i) within its unit
        row_of = [np.zeros((unit_sizes[u], N), np.int64) for u in range(NU)]
        for u in range(NU):
            gu = unit_sizes[u]
            bl = np.arange(gu)
            row_of[u][:, :R] = bl[:, None] * R + np.arange(R)[None, :]
            for l in range(1, L + 1):
                C = int(S_pad[u, l]) // 128
                for t in range(T):
                    bb, ii = nodes[m][u][l][t]
                    s = q_off[u, l, t] + np.arange(len(bb))
                    # partition-major rows: slot s -> row (s%128)*C + s//128
                    row_of[u][bb, ii] = B_ul[u, l] + (s % 128) * C + s // 128
        arr = np.zeros((128, W16), np.int16)
        for u in range(NU):
            g0 = m * BL + ubase[u]
            for l in range(1, L + 1):
                S = int(S_pack[u, l])
                if S == 0:
                    continue
                gv = np.zeros(2 * S, np.int64)
                for t in range(T):
                    bb, ii = nodes[m][u][l][t]
                    o = int(o_off[u, l, t])
                    s = o + np.arange(len(bb))
                    gv[s] = row_of[u][bb, idx_np[g0 + bb, ii, 0]]
                    gv[S + s] = row_of[u][bb, idx_np[g0 + bb, ii, 1]]
                arr[:, gcol[u, l] : gcol[u, l] + (2 * S) // 16] = _wrap16(
                    gv, (2 * S) // 16
                )
        idx16_per_core.append(arr)

    return dict(
        L=L,
        BL=BL,
        NU=NU,
        unit_sizes=unit_sizes,
        ubase=ubase,
        maxcnt=maxcnt,
        o_off=o_off,
        q_off=q_off,
        S_pack=S_pack,
        S_pad=S_pad,
        B_ul=B_ul,
        rows_u=rows_u,
        root_rows=root_rows,
        gcol=gcol,
        W16=W16,
        idx16_per_core=idx16_per_core,
        nodes=nodes,
    )


def build_inputs(root_embeddings, W1, b1, W2, b2, sched, N, R, D, T, M):
    import ml_dtypes

    bf = ml_dtypes.bfloat16
    BL = sched["BL"]
    NU = sched["NU"]
    unit_sizes = sched["unit_sizes"]
    ubase = sched["ubase"]
    W1n = np.asarray(W1, np.float32)  # [T, 128, 128]
    # w1a[t] = [W1[t][0:64,:]; 0], w1b[t] = [W1[t][64:128,:]; 0]
    w1a = np.zeros((128, T * 128), np.float32)
    w1b = np.zeros((128, T * 128), np.float32)
    for t in range(T):
        w1a[0:64, t * 128 : (t + 1) * 128] = W1n[t, 0:64, :]
        w1b[0:64, t * 128 : (t + 1) * 128] = W1n[t, 64:128, :]
    w2 = np.ascontiguousarray(
        np.transpose(np.asarray(W2, np.float32), (1, 0, 2)).reshape(128, T * D)
    )
    b1a = np.ascontiguousarray(np.asarray(b1, np.float32).T)  # [128, T]
    b2f = np.broadcast_to(
        np.asarray(b2, np.float32).reshape(1, T * D), (128, T * D)
    ).copy()

    roots_np = np.asarray(root_embeddings, np.float32)
    in_maps = []
    for m in range(M):
        roots3 = np.zeros((2304, D), np.float32)
        pos = 0
        for u in range(NU):
            gu = unit_sizes[u]
            g0 = m * BL + ubase[u]
            blk = roots_np[g0 : g0 + gu].reshape(gu * R, D)
            roots3[pos : pos + gu * R] = blk
            pos += 768
        in_maps.append(
            dict(
                roots3=roots3,
                w1a=w1a.astype(bf),
                w1b=w1b.astype(bf),
                w2=w2.astype(bf),
                b1a=b1a,
                b2f=b2f,
                idx16=sched["idx16_per_core"][m],
            )
        )
    return in_maps


def _cap_waits(nc, max_waits=1):
    import concourse.mybir as mb

    k = 0
    for f in nc.m.functions:
        for bb in f.blocks:
            out = []
            for ins in bb.instructions:
                si = getattr(ins, "sync_info", None)
                if si is not None and si.on_wait and len(si.on_wait) > max_waits:
                    waits = list(si.on_wait)
                    keep = waits[:max_waits]
                    for wv in waits[max_waits:]:
                        nop = mb.InstNoOp(name=f"waitnop_{k}", ins=[], outs=[])
                        k += 1
                        nop.engine = ins.engine
                        nop.sync_info = mb.SyncInfo(on_wait=[wv], on_update=[])
                        out.append(nop)
                    ins.sync_info = mb.SyncInfo(
                        on_wait=keep, on_update=list(si.on_update or [])
                    )
                out.append(ins)
            bb.instructions = out
    return k


def build_program(sched, N, R, D, T, split_waits=True, zero_b1=True, zero_b2=True):
    import concourse.bass as bass
    import concourse.mybir as mybir
    from concourse import tile
    from concourse.tile_rust import add_dep_helper

    L = sched["L"]
    NU = sched["NU"]
    unit_sizes = sched["unit_sizes"]
    maxcnt = sched["maxcnt"]
    o_off = sched["o_off"]
    q_off = sched["q_off"]
    S_pack = sched["S_pack"]
    S_pad = sched["S_pad"]
    B_ul = sched["B_ul"]
    rows_u = sched["rows_u"]
    root_rows = sched["root_rows"]
    gcol = sched["gcol"]
    W16 = sched["W16"]
    f32 = mybir.dt.float32
    bf16 = mybir.dt.bfloat16
    i16 = mybir.dt.int16

    nc = bass.Bass()
    roots3 = nc.declare_dram_parameter("roots3", [2304, D], f32, isOutput=False)
    w1a = nc.declare_dram_parameter("w1a", [128, T * 128], bf16, isOutput=False)
    w1b = nc.declare_dram_parameter("w1b", [128, T * 128], bf16, isOutput=False)
    w2 = nc.declare_dram_parameter("w2", [128, T * D], bf16, isOutput=False)
    b1a = nc.declare_dram_parameter("b1a", [128, T], f32, isOutput=False)
    b2f = nc.declare_dram_parameter("b2f", [128, T * D], f32, isOutput=False)
    idx16 = nc.declare_dram_parameter("idx16", [128, W16], i16, isOutput=False)
    bufs = [
        nc.declare_dram_parameter(f"buf{u}", [rows_u[u], 128], bf16, isOutput=True)
        for u in range(NU)
    ]

    Spack_max = [int(S_pack[u, 1:].max()) for u in range(NU)]
    Cmax = [int(S_pad[u, 1:].max()) // 128 for u in range(NU)]

    with tile.TileContext(nc) as tc:
        with (
            tc.tile_pool(name="const", bufs=1) as constp,
            tc.tile_pool(name="lvl", bufs=2) as lvlp,
            tc.tile_pool(name="hts", bufs=3) as htsp,
            tc.tile_pool(name="ps", bufs=3, space="PSUM") as psp,
            tc.tile_pool(name="pso", bufs=2, space="PSUM") as psop,
        ):
            from concourse import library_config

            nc.gpsimd.load_library(library_config.mlp)
            w1asb = constp.tile([128, T * 128], bf16)
            nc.sync.dma_start(out=w1asb[:], in_=w1a[:])
            w1bsb = constp.tile([128, T * 128], bf16)
            nc.sync.dma_start(out=w1bsb[:], in_=w1b[:])
            w2sb = constp.tile([128, T * D], bf16)
            nc.sync.dma_start(out=w2sb[:], in_=w2[:])
            b1sb = constp.tile([128, T], f32)
            nc.sync.dma_start(out=b1sb[:], in_=b1a[:])
            b2fsb = constp.tile([128, T * D], f32)
            nc.sync.dma_start(out=b2fsb[:], in_=b2f[:])
            idxsb = constp.tile([128, W16], i16)
            nc.sync.dma_start(out=idxsb[:], in_=idx16[:])

            # roots: [2304,64] f32 -> SBUF -> bf16 -> per-unit payload store
            rsb = constp.tile([128, 18, D], f32)
            nc.sync.dma_start(
                out=rsb[:], in_=roots3[:].rearrange("(c p) d -> p c d", p=128)
            )
            rsbb = constp.tile([128, 18, D], bf16)
            nc.vector.tensor_copy(
                rsbb[:].rearrange("p c d -> p (c d)"),
                rsb[:].rearrange("p c d -> p (c d)"),
            )
            root_stores = []
            for u in range(NU):
                nc_cols = root_rows[u] // 128
                rs = nc.sync.dma_start(
                    out=bufs[u][0 : root_rows[u], 0:D].rearrange(
                        "(c p) d -> p c d", p=128
                    ),
                    in_=rsbb[:, 6 * u : 6 * u + nc_cols, :],
                )
                root_stores.append(rs)

            _regcache = {}

            def creg_for(v):
                if v not in _regcache:
                    _regcache[v] = nc.gpsimd.to_reg(v)
                return _regcache[v]

            prev_stores = [[rs] for rs in root_stores]
            prev_gather = [None] * NU

            for l in range(1, L + 1):
                xs = {}
                gathers = {}
                for u in range(NU):
                    S = int(S_pack[u, l])
                    if S == 0:
                        continue
                    x = lvlp.tile([128, 2 * Spack_max[u]], bf16, tag=f"x{u}")
                    xs[u] = (x, S)
                    g = nc.gpsimd.dma_gather(
                        out_ap=x[:, 0 : 2 * S].rearrange(
                            "p (a s) -> p a s", a=1
                        ),
                        in_ap=bufs[u][:],
                        idxs_ap=idxsb[:, gcol[u, l] : gcol[u, l] + (2 * S) // 16],
                        num_idxs=2 * S,
                        num_idxs_reg=creg_for(2 * S),
                        elem_size=128,
                        transpose=True,
                        single_packet=False,
                    )
                    for ps_h in prev_stores[u]:
                        add_dep_helper(
                            g.ins, ps_h.ins, sync=True, reason="lvl order"
                        )
                    if prev_gather[u] is not None:
                        add_dep_helper(
                            g.ins, prev_gather[u].ins, sync=True, reason="chain"
                        )
                    prev_gather[u] = g
                    gathers[u] = g

                for u in range(NU):
                    if u not in xs:
                        continue
                    x, S = xs[u]
                    SP = int(S_pad[u, l])
                    C = SP // 128
                    osb = lvlp.tile([128, Cmax[u] * D], bf16, tag=f"osb{u}")
                    lvl_stores = []
                    buf_blk = bufs[u][
                        int(B_ul[u, l]) : int(B_ul[u, l]) + C * 128, 0:D
                    ].rearrange("(p c) d -> p c d", c=C)

                    # segments: (t, o, q, n) with n = maxcnt
                    segs = [
                        (
                            t,
                            int(o_off[u, l, t]),
                            int(q_off[u, l, t]),
                            int(maxcnt[u, l, t]),
                        )
                        for t in range(T)
                        if maxcnt[u, l, t] > 0
                    ]
                    q_end = segs[-1][2] + segs[-1][3]

                    # process in hT groups of HT_GROUP padded cols
                    # (HT_GROUP == OPS_CHUNKS*128 so one o_ps tile per group)
                    ngr = -(-q_end // HT_GROUP)
                    for gi in range(ngr):
                        g0 = gi * HT_GROUP
                        g1 = min(g0 + HT_GROUP, q_end)
                        span = g1 - g0
                        hT_ps = psp.tile([128, HT_GROUP], f32, tag="hT")
                        hTs = htsp.tile([128, HT_GROUP], bf16, tag="hTs")
                        o_ps = psop.tile([128, OPS_CHUNKS * D], f32, tag="o")
                        # MM1: per segment piece within [g0,g1), split at
                        # 512 lines (PSUM bank) in local col space
                        for t, o, q, n in segs:
                            lo = max(q, g0)
                            hi = min(q + n, g1)
                            while lo < hi:
                                nxt = min(hi, g0 + ((lo - g0) // 512 + 1) * 512)
                                a = lo - g0
                                b = nxt - g0
                                po = o + (lo - q)
                                pn = nxt - lo
                                nc.tensor.matmul(
                                    hT_ps[:, a:b],
                                    lhsT=w1asb[:, t * 128 : (t + 1) * 128],
                                    rhs=x[:, po : po + pn],
                                    start=True,
                                    stop=False,
                                )
                                nc.tensor.matmul(
                                    hT_ps[:, a:b],
                                    lhsT=w1bsb[:, t * 128 : (t + 1) * 128],
                                    rhs=x[:, S + po : S + po + pn],
                                    start=False,
                                    stop=True,
                                )
                                lo = nxt
                        # gelu over the whole group span (pads = junk)
                        if zero_b1:
                            nc.scalar.activation(
                                hTs[:, 0:span],
                                hT_ps[:, 0:span],
                                mybir.ActivationFunctionType.Gelu,
                                bias=b1sb[:, 0:1],
                            )
                        else:
                            for t, o, q, n in segs:
                                lo = max(q, g0)
                                hi = min(q + n, g1)
                                if lo < hi:
                                    nc.scalar.activation(
                                        hTs[:, lo - g0 : hi - g0],
                                        hT_ps[:, lo - g0 : hi - g0],
                                        mybir.ActivationFunctionType.Gelu,
                                        bias=b1sb[:, t : t + 1],
                                    )
                        # MM2: pieces within this group into o_ps; base
                        # partition tiling: 0 -> any, 64 -> <=64,
                        # 32/96 -> <=32 (PE tile grid)
                        for t, o, q, n in segs:
                            lo = max(q, g0)
                            hi = min(q + n, g1)
                            while lo < hi:
                                c = lo // 128
                                a = lo % 128
                                nb = 64 if a == 32 else 128
                                nxt = min(hi, c * 128 + nb)
                                pn = nxt - lo
                                cc = c % OPS_CHUNKS
                                nc.tensor.matmul(
                                    o_ps[a : a + pn, cc * D : (cc + 1) * D],
                                    lhsT=hTs[:, lo - g0 : nxt - g0],
                                    rhs=w2sb[:, t * D : (t + 1) * D],
                                    start=True,
                                    stop=True,
                                )
                                lo = nxt
                        # drain this group's o_ps -> osb (bf16)
                        c0 = g0 // 128
                        nch = min(-(-span // 128), C - c0)
                        if zero_b2:
                            nc.vector.tensor_copy(
                                osb[:, c0 * D : (c0 + nch) * D],
                                o_ps[:, 0 : nch * D],
                            )
                        else:
                            for t, o, q, n in segs:
                                lo = max(q, g0)
                                hi = min(q + n, g1)
                                while lo < hi:
                                    c = lo // 128
                                    a = lo % 128
                                    nxt = min(hi, (c + 1) * 128)
                                    pn = nxt - lo
                                    cc = c % OPS_CHUNKS
                                    nc.vector.tensor_tensor(
                                        out=osb[a : a + pn, c * D : c * D + D],
                                        in0=o_ps[
                                            a : a + pn, cc * D : cc * D + D
                                        ],
                                        in1=b2fsb[
                                            a : a + pn, t * D : (t + 1) * D
                                        ],
                                        op=mybir.AluOpType.add,
                                    )
                                    lo = nxt
                    # one store per level over the real chunks
                    nreal = -(-q_end // 128)
                    st = nc.sync.dma_start(
                        out=buf_blk[:, 0:nreal, :],
                        in_=osb[:, 0 : nreal * D].rearrange(
                            "p (c d) -> p c d", d=D
                        ),
                    )
                    add_dep_helper(
                        st.ins, gathers[u].ins, sync=True, reason="war"
                    )
                    lvl_stores.append(st)
                    prev_stores[u] = lvl_stores

    from concourse.library_overlay import lower_extended_insts

    lower_extended_insts(nc)
    if split_waits:
        _cap_waits(nc)
    return nc


def assemble_output(results, sched, roots_np, N, R, D, M):
    L = sched["L"]
    NU = sched["NU"]
    BL = sched["BL"]
    unit_sizes = sched["unit_sizes"]
    ubase = sched["ubase"]
    q_off = sched["q_off"]
    B_ul = sched["B_ul"]
    nodes = sched["nodes"]
    out = np.zeros((M * BL, N, D), np.float32)
    out[:, :R] = roots_np
    for m in range(M):
        for u in range(NU):
            buf = np.asarray(results[m][f"buf{u}"]).astype(np.float32)
            g0 = m * BL + ubase[u]
            for l in range(1, L + 1):
                C = int(sched["S_pad"][u, l]) // 128
                for t in range(8):
                    bb, ii = nodes[m][u][l][t]
                    if len(bb) == 0:
                        continue
                    s = int(q_off[u, l, t]) + np.arange(len(bb))
                    rows = int(B_ul[u, l]) + (s % 128) * C + s // 128
                    out[g0 + bb, ii] = buf[rows, 0:D]
    return out


def kernel(**inputs):
    import numpy as np

    root_embeddings = np.asarray(inputs["root_embeddings"], np.float32)
    W1 = np.asarray(inputs["W1"], np.float32)
    b1 = np.asarray(inputs["b1"], np.float32)
    W2 = np.asarray(inputs["W2"], np.float32)
    b2 = np.asarray(inputs["b2"], np.float32)
    idx = np.asarray(inputs["node_inputs_indices"], np.int32)
    types = np.asarray(inputs["node_types"], np.int32)

    B, N, R, D, T, M = B_, N_, R_, D_, T_, M_
    sched = build_schedule(idx, types, B, N, R, T, M)
    in_maps = build_inputs(root_embeddings, W1, b1, W2, b2, sched, N, R, D, T, M)
    nc = build_program(
        sched, N, R, D, T,
        zero_b1=not np.any(b1),
        zero_b2=not np.any(b2),
    )

    from concourse.bass_utils import run_bass_kernel_spmd

    res = run_bass_kernel_spmd(nc, in_maps, core_ids=list(range(M)))
    out = assemble_output(res.results, sched, root_embeddings, N, R, D, M)
    return out.astype(np.float32)


# revision 49
# speedup vs baseline: 1.3034x; 1.2829x over previous
"""Trainium2 Bass kernel for nn_DagnabbitAutoEncoder (gnn_message_passing).

Self-contained: kernel(**inputs) takes FULL inputs, returns FULL [B,N,D]
output. Data-parallel over graphs across 8 NeuronCores; the DAG scan is
converted into ~24 level-wavefronts on the host.

v3 design (vs the scatter-based v2 baseline):
- The node-embedding buffer lives in DRAM as bf16 rows of 256B stride:
  [e (64 bf16) ; zeros (64 bf16)], laid out LEVEL-SORTED so every level's
  outputs form one contiguous block -> the "scatter" becomes a plain
  HWDGE dma_start on SP (cheap), eliminating all scatter descriptors.
- Parent fetch uses the TRANSPOSED dma_gather (elem=128 bf16 = 256B):
  fetch k lands feature-major in column k of X [128, 2S] (parent0 cols
  [0,S), parent1 cols [S,2S)). This kills the PE transpose, the f32->bf16
  cast and the xT PSUM drain of the classic pipeline.
- MM1 = two accumulating matmuls per type segment with half-zeroed
  weights (lhsT [W1a;0] on X0-cols, [W1b;0] on X1-cols) since each
  gathered column only has valid features on partitions 0..63 (the junk
  half of each 256B row is runtime-zeroed DRAM x zero weights).
- Type segments are padded to 64 slots (32 fallback if rows overflow
  int16; matmul output base partition must be 0/32/64/96), rows/osb use
  this padded layout with PARTITION-MAJOR rows (slot s -> row
  (s%128)*C + s//128) so each store partition writes one contiguous
  >=512B run (no small-descriptor DMA penalty). Per-(u,l,t) counts are
  max'd over the 8 cores so the program is SPMD-shared; per-core
  shortfall slots fetch row 0.
"""

B_, N_, R_, D_, K_, T_, M_ = 256, 2048, 64, 64, 2, 8, 8

import numpy as np

UNIT_SIZES = [11, 11, 10]
HT_GROUP = 1024  # hT tile cols (2 PSUM banks); MM pieces split at 512 lines
OPS_CHUNKS = 8  # o_ps tile holds 8 chunks of 64 cols (1 bank)


def compute_levels(idx, R, N):
    B = idx.shape[0]
    lvl = np.zeros((B, N), np.int32)
    ar = np.arange(B)
    for i in range(R, N):
        lvl[:, i] = 1 + lvl[ar[:, None], idx[:, i, :]].max(axis=1)
    return lvl


def _wrap16(vals, ncols):
    """vals [n] -> [128, ncols] int16 wrapped-16 + replicated layout."""
    n = len(vals)
    arr = np.zeros((16, ncols), np.int16)
    k = np.arange(n)
    arr[k % 16, k // 16] = vals
    return np.tile(arr, (8, 1))


def _r(x, m):
    return -(-x // m) * m


def _balance_graphs(Gf, M, unit_sizes, iters=150000, seed=0):
    """Assign graphs to (core, unit) buckets minimizing sum over
    (unit, level, type) of max-over-cores bucket counts (the SPMD padding).
    Returns gmap[m][u] = array of global graph ids."""
    NU = len(unit_sizes)
    NBK = M * NU
    Bg, F = Gf.shape
    rng = np.random.default_rng(seed)
    assign = np.zeros(Bg, np.int32)
    gi = 0
    for m in range(M):
        for u in range(NU):
            assign[gi : gi + unit_sizes[u]] = m * NU + u
            gi += unit_sizes[u]
    S = np.zeros((NBK, F), np.int64)
    np.add.at(S, assign, Gf)

    def ucost(u):
        return S[u::NU].max(axis=0).sum()

    costs = np.array([ucost(u) for u in range(NU)], np.int64)
    for _ in range(iters):
        g1, g2 = rng.integers(0, Bg, 2)
        b1, b2 = assign[g1], assign[g2]
        if b1 == b2:
            continue
        u1, u2 = b1 % NU, b2 % NU
        d = Gf[g2] - Gf[g1]
        S[b1] += d
        S[b2] -= d
        nc1, nc2 = ucost(u1), (ucost(u2) if u2 != u1 else 0)
        oc1, oc2 = costs[u1], (costs[u2] if u2 != u1 else 0)
        if nc1 + nc2 <= oc1 + oc2:
            assign[g1], assign[g2] = b2, b1
            costs[u1] = nc1
            if u2 != u1:
                costs[u2] = nc2
        else:
            S[b1] -= d
            S[b2] += d
    return [
        [np.nonzero(assign == m * NU + u)[0] for u in range(NU)]
        for m in range(M)
    ]


def build_schedule(idx, types, B, N, R, T, M, pad=64):
    BL = B // M
    unit_sizes = UNIT_SIZES
    NU = len(unit_sizes)
    ubase = np.concatenate([[0], np.cumsum(unit_sizes)])
    lvl = compute_levels(idx, R, N)
    L = int(lvl[:, R:].max())
    types_np = np.asarray(types)
    idx_np = np.asarray(idx)

    # balance graphs across (core, unit) buckets to shrink max-over-cores
    Gf = np.zeros((B, (L + 1) * T), np.int64)
    for g in range(B):
        np.add.at(
            Gf[g], lvl[g, R:].astype(np.int64) * T + types_np[g, R:], 1
        )
    gmap = _balance_graphs(Gf, M, unit_sizes)

    # per (core, unit, level, type): node lists (in (b_local, i) order)
    nodes = [[[[None] * T for _ in range(L + 1)] for _ in range(NU)] for _ in range(M)]
    cnt = np.zeros((M, NU, L + 1, T), np.int64)
    for m in range(M):
        for u in range(NU):
            gl = gmap[m][u]
            lv = lvl[gl, R:]
            tp = types_np[gl, R:]
            for l in range(1, L + 1):
                for t in range(T):
                    bb, ii = np.nonzero((lv == l) & (tp == t))
                    nodes[m][u][l][t] = (bb, ii + R)
                    cnt[m, u, l, t] = len(bb)
    maxcnt = cnt.max(axis=0)  # [NU, L+1, T]

    # shared geometry per (u, l): packed offsets o_t, 32-padded offsets q_t
    o_off = np.zeros((NU, L + 1, T), np.int64)
    q_off = np.zeros((NU, L + 1, T), np.int64)
    S_pack = np.zeros((NU, L + 1), np.int64)  # gather slots (64-rounded)
    S_pad = np.zeros((NU, L + 1), np.int64)  # osb/row slots (128-rounded)
    for u in range(NU):
        for l in range(1, L + 1):
            o = q = 0
            for t in range(T):
                o_off[u, l, t] = o
                q_off[u, l, t] = q
                o += maxcnt[u, l, t]
                q += _r(int(maxcnt[u, l, t]), pad)
            S_pack[u, l] = _r(o, 64)
            S_pad[u, l] = _r(q, 128)

    # row bases: root block (128-rounded) then level blocks
    root_rows = [_r(unit_sizes[u] * R, 128) for u in range(NU)]
    B_ul = np.zeros((NU, L + 1), np.int64)
    rows_u = []
    for u in range(NU):
        r = root_rows[u]
        for l in range(1, L + 1):
            B_ul[u, l] = r
            r += S_pad[u, l]
        rows_u.append(int(r))
    if max(rows_u) > 32767 and pad > 32:
        return build_schedule(idx, types, B, N, R, T, M, pad=32)
    assert max(rows_u) <= 32767, f"rows {rows_u} exceed int16"

    # per-core node->row map and gather idx streams; small levels also get
    # scatter idx streams (stored via Pool scatter-add to shorten the
    # store->gather dependency hop)
    SCAT_MAX = 1280
    gcol = np.zeros((NU, L + 1), np.int64)
    scol = np.full((NU, L + 1), -1, np.int64)
    w_after = np.zeros(L + 1, np.int64)
    w = 0
    for l in range(1, L + 1):
        for u in range(NU):
            gcol[u, l] = w
            w += (2 * int(S_pack[u, l])) // 16
            if 0 < S_pad[u, l] <= SCAT_MAX:
                scol[u, l] = w
                w += int(S_pad[u, l]) // 16
        w_after[l] = w
    W16 = max(int(w), 1)
    # idx upload split points (levels 1 | 2-4 | rest) so early gathers
    # don't wait for the whole idx tensor
    idx_cuts = (int(w_after[1]), int(w_after[min(4, L)]))

    idx16_per_core = []
    for m in range(M):
        # row of node (b_local, i) within its unit
        row_of = [np.zeros((unit_sizes[u], N), np.int64) for u in range(NU)]
        for u in range(NU):
            gu = unit_sizes[u]
            bl = np.arange(gu)
            row_of[u][:, :R] = bl[:, None] * R + np.arange(R)[None, :]
            for l in range(1, L + 1):
                C = int(S_pad[u, l]) // 128
                for t in range(T):
                    bb, ii = nodes[m][u][l][t]
                    s = q_off[u, l, t] + np.arange(len(bb))
                    # partition-major rows: slot s -> row (s%128)*C + s//128
                    row_of[u][bb, ii] = B_ul[u, l] + (s % 128) * C + s // 128
        arr = np.zeros((128, W16), np.int16)
        for u in range(NU):
            gl = gmap[m][u]
            for l in range(1, L + 1):
                S = int(S_pack[u, l])
                if S == 0:
                    continue
                gv = np.zeros(2 * S, np.int64)
                for t in range(T):
                    bb, ii = nodes[m][u][l][t]
                    o = int(o_off[u, l, t])
                    s = o + np.arange(len(bb))
                    gv[s] = row_of[u][bb, idx_np[gl[bb], ii, 0]]
                    gv[S + s] = row_of[u][bb, idx_np[gl[bb], ii, 1]]
                arr[:, gcol[u, l] : gcol[u, l] + (2 * S) // 16] = _wrap16(
                    gv, (2 * S) // 16
                )
                if scol[u, l] >= 0:
                    SP_ = int(S_pad[u, l])
                    C = SP_ // 128
                    s = np.arange(SP_)
                    sv = B_ul[u, l] + (s % 128) * C + s // 128
                    arr[:, scol[u, l] : scol[u, l] + SP_ // 16] = _wrap16(
                        sv, SP_ // 16
                    )
        idx16_per_core.append(arr)

    return dict(
        L=L,
        BL=BL,
        NU=NU,
        unit_sizes=unit_sizes,
        ubase=ubase,
        maxcnt=maxcnt,
        o_off=o_off,
        q_off=q_off,
        S_pack=S_pack,
        S_pad=S_pad,
        B_ul=B_ul,
        rows_u=rows_u,
        root_rows=root_rows,
        gcol=gcol,
        scol=scol,
        W16=W16,
        idx_cuts=idx_cuts,
        idx16_per_core=idx16_per_core,
        nodes=nodes,
        gmap=gmap,
    )


def build_inputs(root_embeddings, W1, b1, W2, b2, sched, N, R, D, T, M):
    import ml_dtypes

    bf = ml_dtypes.bfloat16
    BL = sched["BL"]
    NU = sched["NU"]
    unit_sizes = sched["unit_sizes"]
    ubase = sched["ubase"]
    W1n = np.asarray(W1, np.float32)  # [T, 128, 128]
    # w1a[t] = [W1[t][0:64,:]; 0], w1b[t] = [W1[t][64:128,:]; 0]
    w1a = np.zeros((128, T * 128), np.float32)
    w1b = np.zeros((128, T * 128), np.float32)
    for t in range(T):
        w1a[0:64, t * 128 : (t + 1) * 128] = W1n[t, 0:64, :]
        w1b[0:64, t * 128 : (t + 1) * 128] = W1n[t, 64:128, :]
    w2 = np.ascontiguousarray(
        np.transpose(np.asarray(W2, np.float32), (1, 0, 2)).reshape(128, T * D)
    )
    b1a = np.ascontiguousarray(np.asarray(b1, np.float32).T)  # [128, T]
    b2f = np.broadcast_to(
        np.asarray(b2, np.float32).reshape(1, T * D), (128, T * D)
    ).copy()

    roots_np = np.asarray(root_embeddings, np.float32)
    root_rows = sched["root_rows"]
    root_pos = np.concatenate([[0], np.cumsum(root_rows)])
    RTOT = int(root_pos[-1])
    assert RTOT % 128 == 0
    in_maps = []
    for m in range(M):
        roots3 = np.zeros((RTOT, D), np.float32)  # cast to bf16 below
        for u in range(NU):
            gu = unit_sizes[u]
            blk = roots_np[sched["gmap"][m][u]].reshape(gu * R, D)
            roots3[root_pos[u] : root_pos[u] + gu * R] = blk
        in_maps.append(
            dict(
                roots3=roots3.astype(bf),
                w1a=w1a.astype(bf),
                w1b=w1b.astype(bf),
                w2=w2.astype(bf),
                b1a=b1a,
                b2f=b2f,
                idx16=sched["idx16_per_core"][m],
            )
        )
    return in_maps


def _cap_waits(nc, max_waits=1):
    import concourse.mybir as mb

    k = 0
    for f in nc.m.functions:
        for bb in f.blocks:
            out = []
            for ins in bb.instructions:
                si = getattr(ins, "sync_info", None)
                if si is not None and si.on_wait and len(si.on_wait) > max_waits:
                    waits = list(si.on_wait)
                    keep = waits[:max_waits]
                    for wv in waits[max_waits:]:
                        nop = mb.InstNoOp(name=f"waitnop_{k}", ins=[], outs=[])
                        k += 1
                        nop.engine = ins.engine
                        nop.sync_info = mb.SyncInfo(on_wait=[wv], on_update=[])
                        out.append(nop)
                    ins.sync_info = mb.SyncInfo(
                        on_wait=keep, on_update=list(si.on_update or [])
                    )
                out.append(ins)
            bb.instructions = out
    return k


def build_program(sched, N, R, D, T, split_waits=True, zero_b1=True, zero_b2=True):
    import concourse.bass as bass
    import concourse.mybir as mybir
    from concourse import tile
    from concourse.tile_rust import add_dep_helper

    L = sched["L"]
    NU = sched["NU"]
    unit_sizes = sched["unit_sizes"]
    maxcnt = sched["maxcnt"]
    o_off = sched["o_off"]
    q_off = sched["q_off"]
    S_pack = sched["S_pack"]
    S_pad = sched["S_pad"]
    B_ul = sched["B_ul"]
    rows_u = sched["rows_u"]
    root_rows = sched["root_rows"]
    gcol = sched["gcol"]
    scol = sched["scol"]
    W16 = sched["W16"]
    f32 = mybir.dt.float32
    bf16 = mybir.dt.bfloat16
    i16 = mybir.dt.int16

    nc = bass.Bass()
    s3_rows = int(sum(root_rows))
    roots3 = nc.declare_dram_parameter("roots3", [s3_rows, D], bf16, isOutput=False)
    root_cpos = [int(sum(root_rows[:u])) // 128 for u in range(NU + 1)]
    w1a = nc.declare_dram_parameter("w1a", [128, T * 128], bf16, isOutput=False)
    w1b = nc.declare_dram_parameter("w1b", [128, T * 128], bf16, isOutput=False)
    w2 = nc.declare_dram_parameter("w2", [128, T * D], bf16, isOutput=False)
    b1a = nc.declare_dram_parameter("b1a", [128, T], f32, isOutput=False)
    b2f = nc.declare_dram_parameter("b2f", [128, T * D], f32, isOutput=False)
    idx16 = nc.declare_dram_parameter("idx16", [128, W16], i16, isOutput=False)
    bufs = [
        nc.declare_dram_parameter(f"buf{u}", [rows_u[u], 128], bf16, isOutput=True)
        for u in range(NU)
    ]

    Spack_max = [int(S_pack[u, 1:].max()) for u in range(NU)]
    Cmax = [int(S_pad[u, 1:].max()) // 128 for u in range(NU)]

    with tile.TileContext(nc) as tc:
        with (
            tc.tile_pool(name="const", bufs=1) as constp,
            tc.tile_pool(name="lvl", bufs=2) as lvlp,
            tc.tile_pool(name="hts", bufs=4) as htsp,
            tc.tile_pool(name="ps", bufs=3, space="PSUM") as psp,
            tc.tile_pool(name="pso", bufs=2, space="PSUM") as psop,
        ):
            from concourse import library_config

            nc.gpsimd.load_library(library_config.mlp)
            # idx upload split into level-range chunks on SP so level-1
            # gathers start almost immediately; weights/roots go via ACT
            cut1, cut2 = sched["idx_cuts"]
            idxsb = constp.tile([128, W16], i16)
            if cut1 > 0:
                nc.sync.dma_start(
                    out=idxsb[:, 0:cut1], in_=idx16[:, 0:cut1]
                )
            if cut2 > cut1:
                nc.sync.dma_start(
                    out=idxsb[:, cut1:cut2], in_=idx16[:, cut1:cut2]
                )
            if W16 > cut2:
                nc.sync.dma_start(
                    out=idxsb[:, cut2:W16], in_=idx16[:, cut2:W16]
                )
            w1asb = constp.tile([128, T * 128], bf16)
            nc.scalar.dma_start(out=w1asb[:], in_=w1a[:])
            w1bsb = constp.tile([128, T * 128], bf16)
            nc.scalar.dma_start(out=w1bsb[:], in_=w1b[:])
            w2sb = constp.tile([128, T * D], bf16)
            nc.scalar.dma_start(out=w2sb[:], in_=w2[:])
            b1sb = constp.tile([128, T], f32)
            nc.scalar.dma_start(out=b1sb[:], in_=b1a[:])
            b2fsb = constp.tile([128, T * D], f32)
            nc.scalar.dma_start(out=b2fsb[:], in_=b2f[:])

            # roots: host-prepacked bf16, DRAM->DRAM straight into buffer
            # rows (no SBUF stop, no cast) -- unblocks level-1 gathers fast
            root_stores = []
            rpos = 0
            for u in range(NU):
                rs = nc.scalar.dma_start(
                    out=bufs[u][0 : root_rows[u], 0:D],
                    in_=roots3[rpos : rpos + root_rows[u], :],
                )
                root_stores.append(rs)
                rpos += root_rows[u]

            _regcache = {}

            def creg_for(v):
                if v not in _regcache:
                    _regcache[v] = nc.gpsimd.to_reg(v)
                return _regcache[v]

            prev_stores = [[rs] for rs in root_stores]
            prev_gather = [None] * NU

            for l in range(1, L + 1):
                xs = {}
                gathers = {}
                for u in range(NU):
                    S = int(S_pack[u, l])
                    if S == 0:
                        continue
                    x = lvlp.tile([128, 2 * Spack_max[u]], bf16, tag=f"x{u}")
                    xs[u] = (x, S)
                    g = nc.gpsimd.dma_gather(
                        out_ap=x[:, 0 : 2 * S].rearrange(
                            "p (a s) -> p a s", a=1
                        ),
                        in_ap=bufs[u][:],
                        idxs_ap=idxsb[:, gcol[u, l] : gcol[u, l] + (2 * S) // 16],
                        num_idxs=2 * S,
                        num_idxs_reg=creg_for(2 * S),
                        elem_size=128,
                        transpose=True,
                        single_packet=False,
                    )
                    for ps_h in prev_stores[u]:
                        add_dep_helper(
                            g.ins, ps_h.ins, sync=True, reason="lvl order"
                        )
                    if prev_gather[u] is not None:
                        add_dep_helper(
                            g.ins, prev_gather[u].ins, sync=True, reason="chain"
                        )
                    prev_gather[u] = g
                    gathers[u] = g

                for u in range(NU):
                    if u not in xs:
                        continue
                    x, S = xs[u]
                    SP = int(S_pad[u, l])
                    C = SP // 128
                    osb = lvlp.tile([128, Cmax[u] * D], bf16, tag=f"osb{u}")
                    lvl_stores = []
                    buf_blk = bufs[u][
                        int(B_ul[u, l]) : int(B_ul[u, l]) + C * 128, 0:D
                    ].rearrange("(p c) d -> p c d", c=C)

                    # segments: (t, o, q, n) with n = maxcnt
                    segs = [
                        (
                            t,
                            int(o_off[u, l, t]),
                            int(q_off[u, l, t]),
                            int(maxcnt[u, l, t]),
                        )
                        for t in range(T)
                        if maxcnt[u, l, t] > 0
                    ]
                    q_end = segs[-1][2] + segs[-1][3]

                    # MM1 + gelu over PACKED o-space groups (no pad cols on
                    # PE-MM1/ACT); MM2/drains walk the padded q-space and
                    # index back into the packed hTs tiles
                    o_end = segs[-1][1] + segs[-1][3]
                    # o-group boundaries adjusted so their q-positions are
                    # 32-aligned (keeps MM2 base partitions on the PE grid)
                    ogs = []
                    og0 = 0
                    while og0 < o_end:
                        cand = min(og0 + HT_GROUP, o_end)
                        if cand < o_end:
                            # align the boundary's q-position to the 64 grid
                            # (PSUM out base partitions allow only 0/32/64,
                            # and q offsets are 64-aligned)
                            for t, o, q, n in segs:
                                if o <= cand < o + n:
                                    cand -= (q + (cand - o)) % 64
                                    break
                        assert cand > og0
                        ogs.append((og0, cand))
                        og0 = cand
                    hts_tiles = []
                    for og0, og1 in ogs:
                        span = og1 - og0
                        hT_ps = psp.tile([128, HT_GROUP], f32, tag="hT")
                        hTs = htsp.tile([128, HT_GROUP], bf16, tag="hTs")
                        hts_tiles.append(hTs)
                        for t, o, q, n in segs:
                            lo = max(o, og0)
                            hi = min(o + n, og1)
                            while lo < hi:
                                nxt = min(
                                    hi, og0 + ((lo - og0) // 512 + 1) * 512
                                )
                                nc.tensor.matmul(
                                    hT_ps[:, lo - og0 : nxt - og0],
                                    lhsT=w1asb[:, t * 128 : (t + 1) * 128],
                                    rhs=x[:, lo:nxt],
                                    start=True,
                                    stop=False,
                                )
                                nc.tensor.matmul(
                                    hT_ps[:, lo - og0 : nxt - og0],
                                    lhsT=w1bsb[:, t * 128 : (t + 1) * 128],
                                    rhs=x[:, S + lo : S + nxt],
                                    start=False,
                                    stop=True,
                                )
                                lo = nxt
                        if zero_b1:
                            nc.scalar.activation(
                                hTs[:, 0:span],
                                hT_ps[:, 0:span],
                                mybir.ActivationFunctionType.Gelu,
                                bias=b1sb[:, 0:1],
                            )
                        else:
                            for t, o, q, n in segs:
                                lo = max(o, og0)
                                hi = min(o + n, og1)
                                if lo < hi:
                                    nc.scalar.activation(
                                        hTs[:, lo - og0 : hi - og0],
                                        hT_ps[:, lo - og0 : hi - og0],
                                        mybir.ActivationFunctionType.Gelu,
                                        bias=b1sb[:, t : t + 1],
                                    )
                    # MM2 + drains over padded q-space groups; lhsT pieces
                    # split at 128-chunk/base-partition and hTs tile lines
                    ti = 0
                    for gi in range(-(-q_end // HT_GROUP)):
                        g0 = gi * HT_GROUP
                        g1 = min(g0 + HT_GROUP, q_end)
                        o_ps = psop.tile([128, OPS_CHUNKS * D], f32, tag="o")
                        for t, o, q, n in segs:
                            lo = max(q, g0)
                            hi = min(q + n, g1)
                            while lo < hi:
                                c = lo // 128
                                a = lo % 128
                                nb = 64 if a == 32 else 128
                                olo = o + (lo - q)
                                while olo >= ogs[ti][1]:
                                    ti += 1
                                cap = lo + (ogs[ti][1] - olo)
                                nxt = min(hi, c * 128 + nb, cap)
                                pn = nxt - lo
                                cc = c % OPS_CHUNKS
                                loc = olo - ogs[ti][0]
                                nc.tensor.matmul(
                                    o_ps[a : a + pn, cc * D : (cc + 1) * D],
                                    lhsT=hts_tiles[ti][:, loc : loc + pn],
                                    rhs=w2sb[:, t * D : (t + 1) * D],
                                    start=True,
                                    stop=True,
                                )
                                lo = nxt
                        # drain this q-group's o_ps -> osb (bf16)
                        c0 = g0 // 128
                        nch = min(-(-(g1 - g0) // 128), C - c0)
                        if zero_b2:
                            nc.vector.tensor_copy(
                                osb[:, c0 * D : (c0 + nch) * D],
                                o_ps[:, 0 : nch * D],
                            )
                        else:
                            for t, o, q, n in segs:
                                lo = max(q, g0)
                                hi = min(q + n, g1)
                                while lo < hi:
                                    c = lo // 128
                                    a = lo % 128
                                    nxt = min(hi, (c + 1) * 128)
                                    pn = nxt - lo
                                    cc = c % OPS_CHUNKS
                                    nc.vector.tensor_tensor(
                                        out=osb[a : a + pn, c * D : c * D + D],
                                        in0=o_ps[
                                            a : a + pn, cc * D : cc * D + D
                                        ],
                                        in1=b2fsb[
                                            a : a + pn, t * D : (t + 1) * D
                                        ],
                                        op=mybir.AluOpType.add,
                                    )
                                    lo = nxt
                    # store: Pool scatter-add for small levels (keeps the
                    # store->gather dependency on one engine, shortening the
                    # per-level chain), HWDGE dma_start on SP otherwise
                    nreal = -(-q_end // 128)
                    if scol[u, l] >= 0:
                        st = nc.gpsimd.dma_scatter_add(
                            out_ap=bufs[u][:, 0:D],
                            in_ap=osb[:, 0 : C * D].rearrange(
                                "p (c d) -> p c d", d=D
                            ),
                            idxs_ap=idxsb[
                                :, scol[u, l] : scol[u, l] + SP // 16
                            ],
                            num_idxs=SP,
                            num_idxs_reg=creg_for(SP),
                            elem_size=D,
                            elem_step=128,
                            single_packet=False,
                        )
                    else:
                        st = nc.sync.dma_start(
                            out=buf_blk[:, 0:nreal, :],
                            in_=osb[:, 0 : nreal * D].rearrange(
                                "p (c d) -> p c d", d=D
                            ),
                        )
                    add_dep_helper(
                        st.ins, gathers[u].ins, sync=True, reason="war"
                    )
                    lvl_stores.append(st)
                    prev_stores[u] = lvl_stores

    from concourse.library_overlay import lower_extended_insts

    lower_extended_insts(nc)
    if split_waits:
        _cap_waits(nc)
    return nc


def assemble_output(results, sched, roots_np, N, R, D, M):
    L = sched["L"]
    NU = sched["NU"]
    BL = sched["BL"]
    unit_sizes = sched["unit_sizes"]
    ubase = sched["ubase"]
    q_off = sched["q_off"]
    B_ul = sched["B_ul"]
    nodes = sched["nodes"]
    out = np.zeros((M * BL, N, D), np.float32)
    out[:, :R] = roots_np
    for m in range(M):
        for u in range(NU):
            buf = np.asarray(results[m][f"buf{u}"]).astype(np.float32)
            gl = np.asarray(sched["gmap"][m][u])
            for l in range(1, L + 1):
                C = int(sched["S_pad"][u, l]) // 128
                for t in range(8):
                    bb, ii = nodes[m][u][l][t]
                    if len(bb) == 0:
                        continue
                    s = int(q_off[u, l, t]) + np.arange(len(bb))
                    rows = int(B_ul[u, l]) + (s % 128) * C + s // 128
                    out[gl[bb], ii] = buf[rows, 0:D]
    return out


def kernel(**inputs):
    import numpy as np

    root_embeddings = np.asarray(inputs["root_embeddings"], np.float32)
    W1 = np.asarray(inputs["W1"], np.float32)
    b1 = np.asarray(inputs["b1"], np.float32)
    W2 = np.asarray(inputs["W2"], np.float32)
    b2 = np.asarray(inputs["b2"], np.float32)
    idx = np.asarray(inputs["node_inputs_indices"], np.int32)
    types = np.asarray(inputs["node_types"], np.int32)

    B, N, R, D, T, M = B_, N_, R_, D_, T_, M_
    sched = build_schedule(idx, types, B, N, R, T, M)
    in_maps = build_inputs(root_embeddings, W1, b1, W2, b2, sched, N, R, D, T, M)
    nc = build_program(
        sched, N, R, D, T,
        zero_b1=not np.any(b1),
        zero_b2=not np.any(b2),
    )

    from concourse.bass_utils import run_bass_kernel_spmd

    res = run_bass_kernel_spmd(nc, in_maps, core_ids=list(range(M)))
    out = assemble_output(res.results, sched, root_embeddings, N, R, D, M)
    return out.astype(np.float32)
